# revision 22
# baseline (speedup 1.0000x reference)
"""Distributed Trainium2 kernel for fused multi-head attention
(QKV proj + RoPE + causal/key-padded SDPA + out-proj + bias).

Sharding: tensor-parallel over heads across 8 cores (2 heads/core, both
batches on every core).  After each (batch, head) attention pair, one
AllToAll converts that pair's head-shard into sequence-row-shards so the
output projection is computed locally per row slice; the host
concatenates the 8 row slices.  Four small collectives (one per pair)
pipeline through the CC block, so the last one is mostly hidden under
the first batch's projection (the projection accumulates h=0 before
h=1, so it can start before the last exchange lands).

Schedule (PE executes in emission order, so emission order IS the
schedule): QKV for both batches, then attention pairs batch-grouped with
the shorter-L batch first, then projection in the same batch order.

Inside a pair, score tiles are processed in groups of two key tiles
(one 2-bank PSUM tile per group) with a three-group software pipeline:
the next groups' QK matmuls are emitted before the previous group's
AV/denominator matmuls, so the exp activation always overlaps PE work.

Precision: bf16 operands everywhere with fp32 PSUM accumulation and a
fp32 softmax (scores accumulate in fp32, exp reads fp32).  RoPE is
applied in bf16 (the rotated values are stored as bf16 regardless);
the PSUM->SBUF copy that starts it runs on the otherwise-idle Scalar
engine.  The softmax denominator reciprocal uses the fast custom-DVE
approximation (~18 correct bits, ~5x faster than the iterative op).

Key padding is multiplicative instead of additive: V rows and the
denominator ones-vector are zeroed for tokens >= L, so the exp
activation needs no per-key-tile bias and batches over a whole group in
one instruction.  The causal mask is a post-exp affine_select zero
fill; columns below a diagonal tile's valid range hold stale PSUM that
the select also zeroes (matmul moving dims are kept >= 256, where
fp32r/bf16 run at full rate).

Host-side layouts are pre-tiled so every DMA is a contiguous DRAM run
(x: [B, chunk, ktile, 128, 512]; Wproj: [chunk, 128, ktile, 512]; the
output is staged bf16 as [B, m, chunk, 128, 512] and reassembled on the
host).  The x-tile pool holds two full chunks so the next chunk's loads
always run a full chunk ahead of the PE.  Projection weights are loaded
once (they are batch-independent) during the attention phase, when the
DMA queues are otherwise idle.

The kernel is compiled per (ceil(L/128), ceil(L/512)) signature: key
tiles and K/V projection chunks that are entirely masked (k >= L) are
skipped at compile time.  Any L value produces a correct kernel; the
compile cache is keyed on the derived bounds.
"""

import numpy as np
import ml_dtypes

import concourse.bacc as bacc
import concourse.bass as bass
import concourse.mybir as mybir
import concourse.tile as tile
from concourse import bass_utils

B, N, D, NH = 2, 2048, 2048, 16
HD = 128               # head dim
NCORES = 8
HL = NH // NCORES      # heads per core = 2
DL = HL * HD           # local model cols = 256
NS = N // NCORES       # output row slice per core = 256
HALF = HD // 2
ET = D // HD           # 16 contraction tiles
NT = N // HD           # 16 seq tiles of 128
NCH = N // 512         # 4 free-dim chunks of 512
GRP = 2                # key tiles per exp group (2 PSUM banks)
SCALE = 1.0 / float(np.sqrt(HD))
ROPE_BASE = 10000.0

F32 = mybir.dt.float32
F32R = mybir.dt.float32r
BF16 = mybir.dt.bfloat16
I32 = mybir.dt.int32

_CACHE = {}


def _bounds(L):
    """Per-batch compile-time loop bounds from the key-padding lengths."""
    jt = tuple(min(NT, max(1, -(-int(l) // HD))) for l in L)    # key tiles
    kvc = tuple(min(NCH, max(1, -(-int(l) // 512))) for l in L)  # k/v chunks
    return jt, kvc


def build(jtmax, kvcmax):
    key = (jtmax, kvcmax)
    if key in _CACHE:
        return _CACHE[key]
    # process the shorter batch first: its attention pairs are cheaper,
    # so its AllToAlls issue earlier and hide under the longer batch
    border = sorted(range(B), key=lambda b: jtmax[b])

    nc = bacc.Bacc("TRN2", target_bir_lowering=False, debug=False,
                   num_devices=NCORES)
    xTt = nc.dram_tensor("xTt", [B, NCH, HD, ET, 512], BF16,
                         kind="ExternalInput")
    wqkvT = nc.dram_tensor("wqkvT", [3, D, DL], BF16, kind="ExternalInput")
    wpTt = nc.dram_tensor("wpTt", [NCH, HD, ET, 512], BF16,
                          kind="ExternalInput")
    bpbT = nc.dram_tensor("bpbT", [HD, D], BF16, kind="ExternalInput")
    cosT = nc.dram_tensor("cosT", [HD, N], BF16, kind="ExternalInput")
    sinT = nc.dram_tensor("sinT", [HD, N], BF16, kind="ExternalInput")
    mvalT = nc.dram_tensor("mvalT", [HD, B, NT], F32, kind="ExternalInput")
    m128T = nc.dram_tensor("m128T", [HD, B, NT, HD], BF16,
                           kind="ExternalInput")
    out4 = nc.dram_tensor("out4", [B, NS // HD, NCH, HD, 512], BF16,
                          kind="ExternalOutput")

    AF = mybir.ActivationFunctionType
    ALU = mybir.AluOpType

    with tile.TileContext(nc) as tc:
        with tc.tile_pool(name="persist", bufs=1) as pp, \
             tc.tile_pool(name="dram", bufs=1, space="DRAM") as dp:
            # one AllToAll per (batch, head) pair
            ca = [[dp.tile([NCORES, HD, NS], BF16, name=f"ca{b}{h}")
                   for h in range(HL)] for b in range(B)]
            cb = [[dp.tile([NCORES, HD, NS], BF16, name=f"cb{b}{h}")
                   for h in range(HL)] for b in range(B)]

            # projection weights pool opened first so its chunks can be
            # prefetched during attention (it is released last; pool
            # releases must be LIFO)
            pw_ctx = tc.tile_pool(name="projw", bufs=4)
            pw = pw_ctx.__enter__()

            # V stays resident as masked bf16 until the projection phase
            vsctx = tc.tile_pool(name="vres", bufs=1)
            vsp = vsctx.__enter__()
            vsb = [vsp.tile([HD, NT, DL], BF16, name=f"vsb{b}")
                   for b in range(B)]

            # q/k SBUF residency pool, released before the projection
            qk_ctx = tc.tile_pool(name="qkres", bufs=1)
            qkp = qk_ctx.__enter__()
            qsb = [[qkp.tile([HD, N], BF16, name=f"qsb{b}{h}")
                    for h in range(HL)] for b in range(B)]
            ksb = [[qkp.tile([HD, N], BF16, name=f"ksb{b}{h}")
                    for h in range(HL)] for b in range(B)]

            # x pool holds four half-chunks (two full chunks), so the
            # next chunk's two DMAs run a chunk ahead of the matmuls
            xp_ctx = tc.tile_pool(name="ph1x", bufs=4)
            xp = xp_ctx.__enter__()
            wqp_ctx = tc.tile_pool(name="wqkv", bufs=1)
            wqp = wqp_ctx.__enter__()
            wq = wqp.tile([HD, ET, DL], BF16, tag="wq")
            wk = wqp.tile([HD, ET, DL], BF16, tag="wk")
            wv = wqp.tile([HD, ET, DL], BF16, tag="wv")
            # DMA triggers cost ~0.6us of sequencer time each, so loads
            # are batched into few large transfers.  Tiny/early tenants
            # (mask, rope tables) go first; the first x half-chunk and
            # the first half of each weight follow so the first matmuls
            # start ~7us in; the rest stream behind.
            mval = pp.tile([HD, B, NT], F32)
            nc.sync.dma_start(mval[:], mvalT[:])
            xcs0 = []
            cosb = pp.tile([HD, N], BF16)
            sinb = pp.tile([HD, N], BF16)
            # first half-chunk and first weight halves fan out over four
            # engine queues so they transfer concurrently and the first
            # matmul starts as early as possible; tables and second
            # halves stream behind
            # queue plan (transfers on one queue serialize):
            #   sync:   xc-h0, wv-h0, wv-h1          (first MM gate first)
            #   scalar: wq-h0, cos, xc-h1, wq-h1     (xc-h1 ready ~9us,
            #                                         needed ~24us)
            #   gpsimd: wk-h0, sin, wk-h1
            def _wslice(w, i, half):
                hsl = slice(half * (D // 2), (half + 1) * (D // 2))
                return (w[:, half * (ET // 2):(half + 1) * (ET // 2), :],
                        wqkvT[i, hsl].rearrange("(t p) d -> p t d", p=HD))
            for half in range(2):
                xc = xp.tile([HD, ET // 2, 512], BF16, tag="xc",
                             name=f"xc0{half}")
                xcs0.append(xc)
            # gpsimd DMAs use SWDGE (software descriptor generation,
            # ~10us for a 1024-descriptor load) - only sync and scalar
            # have hardware descriptor generation, so everything goes on
            # those two queues
            nc.sync.dma_start(
                xcs0[0][:], xTt[border[0], 0, :, 0:ET // 2, :])
            nc.scalar.dma_start(*_wslice(wq, 0, 0))
            nc.scalar.dma_start(*_wslice(wk, 1, 0))
            nc.sync.dma_start(*_wslice(wv, 2, 0))
            nc.sync.dma_start(*_wslice(wq, 0, 1))
            nc.sync.dma_start(*_wslice(wk, 1, 1))
            nc.sync.dma_start(*_wslice(wv, 2, 1))
            nc.scalar.dma_start(cosb[:], cosT[:])
            nc.scalar.dma_start(sinb[:], sinT[:])
            nc.scalar.dma_start(
                xcs0[1][:], xTt[border[0], 0, :, ET // 2:ET, :])

            # token-validity masks (host-built so nothing on-device gates
            # the v-mask copies): f32 for masking V during the PSUM copy,
            # bf16 replicated across all 128 stationary columns for the
            # denominator matmul (full PE array, no col-group
            # reconfiguration between it and the AV matmul, so LDWEIGHTS
            # pipelines; output already has the denominator in every
            # partition).  mval is tiny and needed first, so it loads
            # before the rope tables / m128 / bias.
            m128 = pp.tile([HD, B, NT, HD], BF16)
            nc.sync.dma_start(m128[:], m128T[:])

            # bias row pre-broadcast to all 128 partitions (bf16)
            bpb = pp.tile([HD, D], BF16)
            nc.sync.dma_start(bpb[:], bpbT[:])


            # ---------------- Phase 1: QKV projection + RoPE ----------------
            with tc.tile_pool(name="ph1s", bufs=3) as sp, \
                 tc.tile_pool(name="ph1p", bufs=2, space="PSUM") as pq:
                for b in border:
                    for c4 in range(NCH):
                        kv = c4 < kvcmax[b]
                        # K columns / V tiles beyond the last valid key
                        # tile are never read by attention (key padding),
                        # so trim the k matmul/rope to kc columns and skip
                        # fully-masked v tiles entirely
                        kc = max(0, min(512, jtmax[b] * HD - c4 * 512))
                        nsl = slice(c4 * 512, (c4 + 1) * 512)
                        ksl = slice(c4 * 512, c4 * 512 + kc)
                        if b == border[0] and c4 == 0:
                            xcs = xcs0
                        else:
                            xcs = []
                            for half in range(2):
                                xc = xp.tile([HD, ET // 2, 512], BF16,
                                             tag="xc", name=f"xc{half}")
                                nc.sync.dma_start(
                                    xc[:],
                                    xTt[b, c4, :, half * (ET // 2):
                                        (half + 1) * (ET // 2), :])
                                xcs.append(xc)
                        # two 4-bank sub-iterations (one per head) so the
                        # PSUM pool double-buffers and the PE never waits
                        # for the rope/copy epilogue
                        for h in range(HL):
                            psq = pq.tile([HD, 512], F32, tag="pq")
                            psk = pq.tile([HD, 512], F32, tag="pk",
                                          name="psk") if kv else None
                            psv = [pq.tile([HD, DL], F32, tag=f"pv{i}",
                                           name=f"psv{i}")
                                   if kv and c4 * 4 + 2 * h + i < jtmax[b]
                                   else None for i in range(2)]
                            for et in range(ET):
                                st = (et == 0)
                                en = (et == ET - 1)
                                xe = xcs[et // (ET // 2)][:, et % (ET // 2), :]
                                nc.tensor.matmul(
                                    psq[:], wq[:, et, h * HD:(h + 1) * HD],
                                    xe, start=st, stop=en)
                                if not kv:
                                    continue
                                nc.tensor.matmul(
                                    psk[:, 0:kc],
                                    wk[:, et, h * HD:(h + 1) * HD],
                                    xe[:, 0:kc], start=st, stop=en)
                                for i in range(2):
                                    if c4 * 4 + 2 * h + i >= jtmax[b]:
                                        continue
                                    s4 = 2 * h + i
                                    nc.tensor.matmul(
                                        psv[i][:],
                                        xe[:, s4 * HD:(s4 + 1) * HD],
                                        wv[:, et, :], start=st, stop=en)
                            # copy-first RoPE: the Scalar engine (idle in
                            # this phase) copies PSUM->bf16 SBUF, freeing
                            # the PSUM bank after one op; the rotation then
                            # runs in bf16 on the Vector engine (tables
                            # half-duplicated so every tensor_tensor has
                            # equal partition bases)
                            pairs = [(psq, qsb[b][h], 512, nsl)]
                            if kv:
                                pairs.append((psk, ksb[b][h], kc, ksl))
                            last_ch = (b == border[-1] and c4 == NCH - 1)
                            for src, dst, w_, sl_ in pairs:
                                stg = sp.tile([HD, 512], BF16, tag="stg")
                                t12 = sp.tile([HD, 512], BF16, tag="t12")
                                if last_ch:
                                    nc.vector.tensor_copy(stg[:, 0:w_],
                                                          src[:, 0:w_])
                                else:
                                    nc.scalar.copy(stg[:, 0:w_],
                                                   src[:, 0:w_])
                                nc.vector.tensor_mul(
                                    t12[:HALF, 0:w_], stg[HALF:, 0:w_],
                                    sinb[HALF:, sl_])
                                nc.vector.tensor_mul(
                                    t12[HALF:, 0:w_], stg[:HALF, 0:w_],
                                    sinb[:HALF, sl_])
                                nc.vector.tensor_mul(stg[:, 0:w_],
                                                     stg[:, 0:w_],
                                                     cosb[:, sl_])
                                nc.vector.tensor_sub(dst[:HALF, sl_],
                                                     stg[:HALF, 0:w_],
                                                     t12[:HALF, 0:w_])
                                nc.vector.tensor_add(dst[HALF:, sl_],
                                                     stg[HALF:, 0:w_],
                                                     t12[HALF:, 0:w_])
                            if kv:
                                for i in range(2):
                                    tb = c4 * 4 + 2 * h + i
                                    if tb >= jtmax[b]:
                                        continue
                                    nc.vector.tensor_scalar(
                                        vsb[b][:, tb, :], psv[i][:],
                                        mval[:, b, tb:tb + 1], None, ALU.mult)
            wqp_ctx.__exit__(None, None, None)
            xp_ctx.__exit__(None, None, None)

            # ------- Phase 2: attention + one AllToAll per (batch, head) ----
            with tc.tile_pool(name="atts", bufs=5) as sp2, \
                 tc.tile_pool(name="attn", bufs=2) as sp3, \
                 tc.tile_pool(name="attp", bufs=2, space="PSUM") as pq2, \
                 tc.tile_pool(name="attpo", bufs=2, space="PSUM") as pq3:

                def qk_group(grp):
                    c4, jts = grp["c4"], grp["jts"]
                    qt, kt = grp["qt"], grp["kt"]
                    pss = pq2.tile([HD, GRP * 512], F32, tag="pss",
                                   name="pss")
                    pt = sp2.tile([HD, GRP * 512], BF16, tag="pt", name="pt")
                    grp["pt"] = pt
                    for j, jt in enumerate(jts):
                        # diagonal tiles (jt = 4*c4+r, r>0) have no valid
                        # columns below 128*r; keep the matmul moving dim
                        # >= 256 (small-moving runs at quarter rate)
                        r = jt - 4 * c4
                        lo = min(128 * r, 256) if r > 0 else 0
                        grp["lo"][jt] = (j, lo, r)
                        nc.tensor.matmul(
                            pss[:, j * 512 + lo:(j + 1) * 512],
                            kt[:, jt * HD:(jt + 1) * HD],
                            qt[:, c4 * 512 + lo:(c4 + 1) * 512],
                            start=True, stop=True)
                    # one exp per group; columns below each tile's lo hold
                    # stale PSUM, exp'd but never read by the AV matmuls
                    lo0 = grp["lo"][jts[0]][1]
                    gw = (grp["lo"][jts[-1]][0] + 1) * 512
                    nc.scalar.activation(pt[:, lo0:gw], pss[:, lo0:gw],
                                         AF.Exp, scale=SCALE)
                    for jt in jts:
                        j, lo, r = grp["lo"][jt]
                        if r >= 0:
                            nc.gpsimd.affine_select(
                                out=pt[:, j * 512 + lo:(j + 1) * 512],
                                in_=pt[:, j * 512 + lo:(j + 1) * 512],
                                compare_op=ALU.is_ge, fill=0.0,
                                base=lo - 128 * r,
                                pattern=[[1, 512 - lo]],
                                channel_multiplier=-1)

                def av_group(grp, state):
                    b, h = grp["b"], grp["h"]
                    c4, jts, njt = grp["c4"], grp["jts"], grp["njt"]
                    vt = grp["vt"]
                    if jts[0] == 0:
                        state["pso"] = pq3.tile([HD, 512], F32, tag="pso",
                                                name="pso")
                        state["psd"] = pq3.tile([HD, 512], F32, tag="psd",
                                                name="psd")
                    pso, psd = state["pso"], state["psd"]
                    pt = grp["pt"]
                    for jt in jts:
                        j, lo, r = grp["lo"][jt]
                        nc.tensor.matmul(
                            pso[:, lo:], vt[:, jt, :],
                            pt[:, j * 512 + lo:(j + 1) * 512],
                            start=(jt == 0), stop=(jt == njt - 1))
                    for jt in jts:
                        j, lo, r = grp["lo"][jt]
                        nc.tensor.matmul(
                            psd[:, lo:], m128[:, b, jt, :],
                            pt[:, j * 512 + lo:(j + 1) * 512],
                            start=(jt == 0), stop=(jt == njt - 1))
                    if jts[-1] == njt - 1:        # last group of this c4
                        rec = sp3.tile([HD, 512], F32, tag="rec", name="rec")
                        nc.vector.reciprocal_approx_fast(rec[:], psd[:])
                        ou = sp3.tile([HD, 512], BF16, tag="ou", name="ou")
                        nc.vector.tensor_mul(ou[:], pso[:], rec[:])
                        nc.sync.dma_start(
                            ca[b][h][2 * c4:2 * c4 + 2].rearrange(
                                "s p n -> p s n"),
                            ou[:].rearrange("p (s n) -> p s n", s=2))

                allg = []
                for b in reversed(border):
                    for h in range(HL):
                        vt = vsb[b][:, :, h * HD:(h + 1) * HD]
                        for c4 in range(NCH):
                            njt = min(4 * c4 + 4, jtmax[b])
                            ngrp = -(-njt // GRP)
                            for g in range(ngrp):
                                jts = list(range(
                                    GRP * g, min(GRP * g + GRP, njt)))
                                allg.append({
                                    "b": b, "h": h, "c4": c4, "jts": jts,
                                    "njt": njt, "lo": {},
                                    "qt": qsb[b][h], "kt": ksb[b][h],
                                    "vt": vt,
                                    "a2a": (c4 == NCH - 1
                                            and jts[-1] == njt - 1),
                                })

                # three-group software pipeline across all pairs: the
                # exp -> causal-mask chain completes well before AV needs
                # the tile, so the PE sequencer's run-ahead is never reset
                # by a just-in-time wait and LDWEIGHTS overlaps the
                # previous matmul (pss frees at exp-read, two bufs suffice)
                state = {}
                wpts = []
                pending = []

                def issue_a2a(b, h):
                    # deferred ~3 groups after the pair completes, so the
                    # collective's wait on the ca stores is already
                    # satisfied when the gpsimd FIFO reaches it (it would
                    # otherwise block the next groups' affine_selects)
                    nc.gpsimd.collective_compute(
                        "AllToAll", mybir.AluOpType.bypass,
                        replica_groups=[list(range(NCORES))],
                        ins=[ca[b][h].opt()], outs=[cb[b][h].opt()])
                    if b == border[-1] and h == 0:
                        # projection weights are batch-independent: load
                        # all four chunks once, now, while the DMA queues
                        # are idle and attention computes
                        for f4 in range(NCH):
                            wpt = pw.tile([HD, ET, 512], BF16,
                                          tag="wpt", name=f"wpt{f4}")
                            nc.sync.dma_start(wpt[:], wpTt[f4])
                            wpts.append(wpt)

                for j0 in range(min(4, len(allg))):
                    qk_group(allg[j0])
                for i, grp in enumerate(allg):
                    if i + 4 < len(allg):
                        qk_group(allg[i + 4])
                    if pending and pending[0][0] <= i:
                        issue_a2a(*pending.pop(0)[1])
                    av_group(grp, state)
                    if grp["a2a"]:
                        pending.append((i + 3, (grp["b"], grp["h"])))
                for _, bh in pending:
                    issue_a2a(*bh)

            qk_ctx.__exit__(None, None, None)
            vsctx.__exit__(None, None, None)

            # ---------------- Phase 3: output projection ----------------
            # batch-major in the same order the exchanges complete; the
            # contraction runs h=0 slots before h=1 slots, so each psp
            # chunk only needs the h=1 exchange for its second half
            with tc.tile_pool(name="proj", bufs=1) as pj, \
                 tc.tile_pool(name="projs", bufs=4) as po, \
                 tc.tile_pool(name="projp", bufs=8, space="PSUM") as pq4:
                # asb[h][p, b, s, n] = cb[b][h][s, p, n]
                asb = [pj.tile([HD, B, NCORES, NS], BF16, name=f"asb{h}")
                       for h in range(HL)]
                for b in reversed(border):
                    for h in range(HL):
                        nc.scalar.dma_start(
                            asb[h][:, b],
                            cb[b][h][:].rearrange("s p n -> p s n"))
                for bi, b in enumerate(reversed(border)):
                    # first batch: all eight chunks' h=0 halves first (its
                    # h=1 exchange may still be in flight), then the h=1
                    # halves; PSUM holds all eight accumulators.  second
                    # batch: exchanges long done, plain per-chunk loop.
                    passes = [(0, 0), (1, 1)]
                    psps = {}
                    for hlo, hhi in passes:
                        for f4 in range(NCH):
                            fsl = slice(f4 * 512, (f4 + 1) * 512)
                            wpt = wpts[f4]
                            for m in range(NS // HD):
                                if (f4, m) in psps:
                                    psp = psps[(f4, m)]
                                else:
                                    psp = pq4.tile([HD, 512], F32, tag="psp",
                                                   name="psp")
                                    psps[(f4, m)] = psp
                                for h in range(hlo, hhi + 1):
                                    for s in range(NCORES):
                                        gi = h * NCORES + s
                                        nc.tensor.matmul(
                                            psp[:],
                                            asb[h][:, b, s,
                                                   m * HD:(m + 1) * HD],
                                            wpt[:, 2 * s + h, :],
                                            start=(gi == 0),
                                            stop=(gi == ET - 1))
                                if hhi == 1:
                                    ot = po.tile([HD, 512], BF16, tag="ot",
                                                 name="ot")
                                    nc.vector.tensor_add(ot[:], psp[:],
                                                         bpb[:, fsl])
                                    nc.sync.dma_start(out4[b, m, f4], ot[:])
                        if hhi == 1:
                            psps = {}

            pw_ctx.__exit__(None, None, None)

    nc.compile()
    _CACHE[key] = nc
    return nc


def _prep_inputs(x, Wqkv, Wproj, bproj, L):
    x = np.asarray(x, np.float32)
    Wqkv = np.asarray(Wqkv, np.float32)
    Wproj = np.asarray(Wproj, np.float32)
    bproj = np.asarray(bproj, np.float32)
    L = np.asarray(L, np.int32)

    xT = x.transpose(0, 2, 1).astype(ml_dtypes.bfloat16)      # [B, D, N]
    xTt = np.ascontiguousarray(
        xT.reshape(B, ET, HD, NCH, 512).transpose(0, 3, 2, 1, 4))
    wpT = Wproj.T.astype(ml_dtypes.bfloat16)                  # [D, D]
    wpTt = np.ascontiguousarray(
        wpT.reshape(ET, HD, NCH, 512).transpose(2, 1, 0, 3))
    inv = 1.0 / (ROPE_BASE ** (np.arange(0, HD, 2, dtype=np.float32) / HD))
    ang = np.arange(N, dtype=np.float32)[:, None] * inv[None, :]
    cos1 = np.cos(ang).T
    sin1 = np.sin(ang).T
    cosT = np.ascontiguousarray(
        np.vstack([cos1, cos1])).astype(ml_dtypes.bfloat16)   # [128, N]
    sinT = np.ascontiguousarray(
        np.vstack([sin1, sin1])).astype(ml_dtypes.bfloat16)
    # host-built masks: mval[p, b, t] = (t*128+p < L[b]); m128 replicates
    # it across the 128 stationary columns of the denominator matmul
    tok = (np.arange(NT)[None, :] * HD
           + np.arange(HD)[:, None])                          # [HD, NT]
    mval = (tok[:, None, :] < L[None, :, None]).astype(np.float32)
    m128 = np.ascontiguousarray(
        np.broadcast_to(mval[:, :, :, None],
                        (HD, B, NT, HD))).astype(ml_dtypes.bfloat16)
    bpbT = np.ascontiguousarray(
        np.broadcast_to(bproj[None, :], (HD, D))).astype(ml_dtypes.bfloat16)

    in_maps = []
    for c in range(NCORES):
        sl = slice(c * DL, (c + 1) * DL)
        w3 = np.stack([
            np.ascontiguousarray(Wqkv[0 * D:1 * D][sl].T),
            np.ascontiguousarray(Wqkv[1 * D:2 * D][sl].T),
            np.ascontiguousarray(Wqkv[2 * D:3 * D][sl].T),
        ]).astype(ml_dtypes.bfloat16)
        in_maps.append({
            "xTt": xTt, "wqkvT": w3, "wpTt": wpTt, "bpbT": bpbT,
            "cosT": cosT, "sinT": sinT, "mvalT": mval, "m128T": m128,
        })
    return in_maps


def run(x, Wqkv, Wproj, bproj, L, trace=False, tmpdir=None):
    jtmax, kvcmax = _bounds(np.asarray(L).reshape(-1))
    nc = build(jtmax, kvcmax)
    in_maps = _prep_inputs(x, Wqkv, Wproj, bproj, L)
    kw = {}
    if tmpdir is not None:
        kw["tmpdir"] = tmpdir
    res = bass_utils.run_bass_kernel_spmd(
        nc, in_maps, core_ids=list(range(NCORES)), trace=trace, **kw)
    full = np.empty((B, N, D), np.float32)
    for c in range(NCORES):
        o4 = np.asarray(res.results[c]["out4"], dtype=np.float32)
        full[:, c * NS:(c + 1) * NS, :] = (
            o4.transpose(0, 1, 3, 2, 4).reshape(B, NS, D))
    return full, res


def kernel(x, Wqkv, Wproj, bproj, L, n_heads):
    assert int(n_heads) == NH
    full, _ = run(x, Wqkv, Wproj, bproj, L, trace=False)
    return full
